# revision 1
# baseline (speedup 1.0000x reference)
"""Trainium2 kernel for nn_NoBrainEncoderBlock_31662498906140.

out = softmax_n( clip( cos(q1_row, k1_row_n) * mask, 0, 1 ) )

Only q1, k1, mask affect the output (q2, k2, temp are unused by the math),
so only those are transferred to the device — halving HBM traffic vs naive.

Sharding: data-parallel over batch B=32 across the 8 NeuronCores (4 rows
per core); all reductions are per-row (over D) or per-row softmax (over N),
so there is no cross-core communication. Executed via jax.pmap on the 8
axon-tunneled cores; XLA/neuronxcc fuses the per-shard computation into a
single on-device program that streams each core's 64 MiB k1 shard.
"""

import numpy as np

B, N, D = 32, 2048, 2048
NCORES = 8
BPC = B // NCORES
NORM_EPS = 1e-12
COS_EPS = 1e-8

_PMAPPED = None


def _get_pmapped():
    global _PMAPPED
    if _PMAPPED is not None:
        return _PMAPPED
    import jax
    import jax.numpy as jnp

    def shard_fn(q1s, k1s, masks):
        # q1s [BPC, D], k1s [BPC, N, D], masks [BPC, N]
        nq = jnp.sqrt(jnp.sum(q1s * q1s, axis=-1, keepdims=True))   # [BPC,1]
        q1n = q1s / jnp.maximum(nq, NORM_EPS)                        # [BPC,D]
        dot = jnp.einsum("bd,bnd->bn", q1n, k1s)                     # [BPC,N]
        nk = jnp.sqrt(jnp.sum(k1s * k1s, axis=-1))                   # [BPC,N]
        nk = jnp.maximum(nk, NORM_EPS)
        scores = dot / jnp.maximum(nk, COS_EPS)
        scores = scores * masks
        scores = jnp.clip(scores, 0.0, 1.0)
        # scores are in [0,1] -> exp is safe without max-subtraction
        e = jnp.exp(scores)
        return e / jnp.sum(e, axis=-1, keepdims=True)

    devices = jax.devices()[:NCORES]
    _PMAPPED = jax.pmap(shard_fn, devices=devices)
    return _PMAPPED


def kernel(q1, k1, q2, k2, mask, temp):
    q1 = np.asarray(q1, dtype=np.float32).reshape(NCORES, BPC, D)
    k1 = np.asarray(k1, dtype=np.float32).reshape(NCORES, BPC, N, D)
    mask = np.asarray(mask, dtype=np.float32).reshape(NCORES, BPC, N)
    fn = _get_pmapped()
    out = fn(q1, k1, mask)
    return np.asarray(out, dtype=np.float32).reshape(B, N)

